# revision 1
# baseline (speedup 1.0000x reference)
"""Trainium2 Bass kernel for MaterialsGraphSAGE (4-layer GraphSAGE + pooling).

Strategy (8 NeuronCores, one chip):
- Node space padded to 50176 = 8 x 6272; core c owns nodes [6272c, 6272(c+1)).
- Edges are owned by their dst core, grouped per 128-node dst block, split by
  src half (dma_gather idx is int16, so the h table is addressed as two 25088
  row halves), padded to 128-edge tiles. Tile counts per (block, half) are
  normalized to the max across cores so the SPMD program structure is
  core-independent; only the idx / dstoff data differs per core.
- Per layer: dma_gather fetches h[src] rows (4 SWDGE queues), the scatter-mean
  is a one-hot matmul accumulated in PSUM (transposed: sumsT[f, n]), per-node
  1/deg scaling uses broadcast tiles (degree computed on device via
  ones-matmuls), then the dense SAGE update + BN runs in transposed layout so
  per-channel affine ops are per-partition. Each core's new h slice is written
  to DRAM and AllGathered into a pair-shared full table for the next layer.
- Final layer skips the table and accumulates graph pooling (one-hot over
  graph ids) + graph counts; contributions ride a small AllGather; every core
  computes the tiny final MLP; core 0's output is returned.
"""

import sys

for _p in ("/opt/trn_rl_repo",):
    if _p not in sys.path:
        sys.path.insert(0, _p)

import numpy as np

import concourse.bacc as bacc
import concourse.mybir as mybir
import concourse.tile as tile
from concourse.bass_utils import run_bass_kernel_spmd
from concourse.vector_clock import ScopedClock

F32 = mybir.dt.float32
BF16 = mybir.dt.bfloat16
I16 = mybir.dt.int16

P = 128
NCORES = 8
NN = 50000
NG = 256
SLICE = 6272
PADN = SLICE * NCORES      # 50176
HALF = PADN // 2           # 25088
NB = SLICE // P            # 49 blocks per core
NL = 4                     # SAGE layers
H = 128
NODE_F = 64
CH_TILES = 16              # gather tiles per dma_gather call
NEG_SLOPE = 0.01
BN_EPS = 1e-5


# ---------------------------------------------------------------------------
# walrus in this container rejects >1 sync wait per instruction; split them.
def _patch_tile_drain():
    def _drain_and_barrier(self, tick_clock, wait_clock):
        drain_inst = self.nc.sync.drain()
        wait_clock.add_sem_waits(
            drain_inst.ins, ScopedClock({None: tick_clock.global_clock})
        )
        si = drain_inst.ins.sync_info
        waits = list(si.on_wait) if si is not None else []
        if len(waits) > 1:
            drain_inst.ins.sync_info = mybir.SyncInfo(
                on_wait=[waits[0]], on_update=list(si.on_update)
            )
            for w in waits[1:]:
                extra = self.nc.sync.drain()
                extra.ins.sync_info = mybir.SyncInfo(on_wait=[w], on_update=[])
        self.nc.all_engine_barrier()
        assert self.sems is not None
        popped = self.nc._tile_sem_poison_stack.pop()
        assert popped is self._sem_poison
        self.nc.clear_and_free_semaphores(list(self.sems.allocated().values()))
        self.nc.all_engine_barrier()

    tile.TileContext._drain_and_barrier = _drain_and_barrier


_patch_tile_drain()


def _legalize_sync_waits(nc, max_waits=1):
    for fn in nc.m.functions:
        for bb in fn.blocks:
            out = []
            changed = False
            for ins in bb.instructions:
                si = ins.sync_info
                if si is not None and len(si.on_wait) > max_waits:
                    waits = list(si.on_wait)
                    for w in waits[:-max_waits]:
                        nop = mybir.InstNoOp(
                            name=f"WSPLIT-{nc.next_id()}", ins=[], outs=[]
                        )
                        nop.engine = ins.engine
                        nop.sync_info = mybir.SyncInfo(on_wait=[w], on_update=[])
                        out.append(nop)
                    ins.sync_info = mybir.SyncInfo(
                        on_wait=waits[-max_waits:], on_update=list(si.on_update)
                    )
                    changed = True
                out.append(ins)
            if changed:
                bb.instructions = out


# ---------------------------------------------------------------------------
def _wrap_idx(flat):
    """int16 row indices -> dma_gather idx buffer [128, n/16] (wrapped in 16
    partitions, replicated across the 8 Q7 core groups)."""
    n = flat.shape[0]
    assert n % 16 == 0
    buf = np.zeros((P, n // 16), np.int16)
    j = np.arange(n)
    for k in range(8):
        buf[16 * k + (j % 16), j // 16] = flat
    return buf


def _prepare(src, dst, batch_gid):
    """Group edges per core / dst block / src half; normalize tile counts
    across cores so all cores share one program structure."""
    per_core = []
    for c in range(NCORES):
        base = c * SLICE
        m = (dst >= base) & (dst < base + SLICE)
        s = src[m]
        d = dst[m]
        blk = (d - base) >> 7
        half = (s >= HALF).astype(np.int64)
        key = blk * 2 + half
        order = np.argsort(key, kind="stable")
        s, d, key = s[order], d[order], key[order]
        bounds = np.searchsorted(key, np.arange(2 * NB + 1))
        cells = {}
        for b in range(NB):
            for h in (0, 1):
                lo, hi = bounds[2 * b + h], bounds[2 * b + h + 1]
                if hi > lo:
                    sl = (s[lo:hi] - (HALF if h else 0)).astype(np.int16)
                    doff = (d[lo:hi] - base - b * P).astype(np.float32)
                    cells[(b, h)] = (sl, doff)
        per_core.append(cells)

    # normalized tile counts
    NT = np.zeros((NB, 2), np.int64)
    for b in range(NB):
        for h in (0, 1):
            n = max((len(per_core[c].get((b, h), ((), ()))[0])
                     for c in range(NCORES)), default=0)
            NT[b, h] = -(-n // P)
        if NT[b].sum() == 0:
            NT[b, 0] = 1

    nt_lo = int(NT[:, 0].sum())
    nt_hi = int(NT[:, 1].sum())

    # shared structure: stream positions and block refs
    pos = {0: 0, 1: 0}
    block_refs = [[] for _ in range(NB)]
    tile_pos = {}              # (b,h,t) -> (stream, stream_pos, gidx)
    for b in range(NB):
        for h in (0, 1):
            for t in range(int(NT[b, h])):
                p_ = pos[h]
                g = p_ if h == 0 else nt_lo + p_
                block_refs[b].append((h, p_ // CH_TILES, p_ % CH_TILES, g))
                tile_pos[(b, h, t)] = (h, p_, g)
                pos[h] += 1

    # chunk sizes per stream (last may be partial)
    chunks = {}
    for h, nt in ((0, nt_lo), (1, nt_hi)):
        chunks[h] = [min(CH_TILES, nt - c0) for c0 in range(0, nt, CH_TILES)]

    # per-core data buffers
    core_data = []
    for c in range(NCORES):
        flat = {0: np.zeros(nt_lo * P, np.int16),
                1: np.zeros(nt_hi * P, np.int16)}
        dstoff = np.full((P, nt_lo + nt_hi), 255.0, np.float32)
        for b in range(NB):
            for h in (0, 1):
                sl, doff = per_core[c].get((b, h), (np.zeros(0, np.int16),
                                                    np.zeros(0, np.float32)))
                n = len(sl)
                for t in range(int(NT[b, h])):
                    _, p_, g = tile_pos[(b, h, t)]
                    seg_s = sl[t * P:(t + 1) * P]
                    seg_d = doff[t * P:(t + 1) * P]
                    flat[h][p_ * P:p_ * P + len(seg_s)] = seg_s
                    dstoff[:len(seg_d), g] = seg_d
        core_data.append(dict(
            idx_lo=_wrap_idx(flat[0]) if nt_lo else np.zeros((P, 8), np.int16),
            idx_hi=_wrap_idx(flat[1]) if nt_hi else np.zeros((P, 8), np.int16),
            dstoff=dstoff,
        ))

    # graph id per node column layout [128, NB] (pad nodes -> -1)
    gids = []
    for c in range(NCORES):
        base = c * SLICE
        col = np.full(SLICE, -1.0, np.float32)
        npad = min(max(NN - base, 0), SLICE)
        if npad > 0:
            col[:npad] = batch_gid[base:base + npad]
        gids.append(col.reshape(NB, P).T.copy())

    return dict(nt_lo=nt_lo, nt_hi=nt_hi, chunks=chunks,
                block_refs=block_refs, core_data=core_data, gids=gids)


def kernel(x, edge_index, u, batch, W_emb, b_emb, W_l, b_l, W_r, gamma, beta,
           W_g, b_g, W_f1, b_f1, W_f2, b_f2):
    x = np.asarray(x, np.float32)
    edge_index = np.asarray(edge_index)
    u = np.asarray(u, np.float32)
    batch = np.asarray(batch)

    src = edge_index[0].astype(np.int64)
    dst = edge_index[1].astype(np.int64)
    prep = _prepare(src, dst, batch.astype(np.float32))

    nt_lo, nt_hi = prep["nt_lo"], prep["nt_hi"]
    chunks = prep["chunks"]
    block_refs = prep["block_refs"]

    xT = np.zeros((NODE_F, PADN), np.float32)
    xT[:, :NN] = x.T

    inv_std = np.float32(1.0 / np.sqrt(1.0 + BN_EPS))
    gscale = np.asarray(gamma, np.float32) * inv_std  # [L, H]
    beta_np = np.asarray(beta, np.float32)

    iota_np = np.broadcast_to(np.arange(P, dtype=np.float32), (P, P)).copy()
    giota_np = np.broadcast_to(np.arange(NG, dtype=np.float32), (P, NG)).copy()
    ident_np = np.eye(P, dtype=np.float32)
    ones_np = np.ones((P, 1), np.float32)

    # ------------------------------------------------------------------
    nc = bacc.Bacc(None, num_swdge_queues=4)

    def din(name, shape, dtype=F32):
        return nc.dram_tensor(name, shape, dtype, kind="ExternalInput")

    xT_in = din("xT", [NODE_F, SLICE])
    idx_lo_in = din("idx_lo", [P, max(nt_lo, 1) * 8], I16)
    idx_hi_in = din("idx_hi", [P, max(nt_hi, 1) * 8], I16)
    dstoff_in = din("dstoff", [P, nt_lo + nt_hi])
    gid_in = din("gid", [P, NB])
    iota_in = din("iota", [P, P])
    giota_in = din("giota", [P, NG])
    ident_in = din("ident", [P, P])
    ones_in = din("ones", [P, 1])
    wemb_in = din("wemb", [NODE_F, H])
    bemb_in = din("bemb", [P, 1])
    wl_in = din("wl", [H, NL * H])
    wr_in = din("wr", [H, NL * H])
    bl_in = din("bl", [P, NL])
    gs_in = din("gs", [P, NL])
    bt_in = din("bt", [P, NL])
    uT_in = din("uT", [16, NG])
    wg_in = din("wg", [16, H])
    bg_in = din("bg", [P, 1])
    wf1_in = din("wf1", [2 * H, H])
    bf1_in = din("bf1", [P, 1])
    wf2_in = din("wf2", [H, 2])
    bf2_in = din("bf2", [2, 1])
    y_out = nc.dram_tensor("y", [2, NG], F32, kind="ExternalOutput")

    RG = [list(range(NCORES))]
    AluOp = mybir.AluOpType
    Act = mybir.ActivationFunctionType

    with tile.TileContext(nc) as tc:
        with (
            tc.tile_pool(name="dram", bufs=1, space="DRAM") as dram,
            tc.tile_pool(name="meta", bufs=1) as meta,
            tc.tile_pool(name="hbuf", bufs=1) as hbuf,
            tc.tile_pool(name="glo", bufs=2) as glo,
            tc.tile_pool(name="ghi", bufs=2) as ghi,
            tc.tile_pool(name="oh", bufs=4) as ohp,
            tc.tile_pool(name="mean", bufs=3) as meanp,
            tc.tile_pool(name="pre", bufs=3) as prep_,
            tc.tile_pool(name="stg", bufs=3) as stgp,
            tc.tile_pool(name="ps_scat", bufs=2, space="PSUM") as ps_scat,
            tc.tile_pool(name="ps_dense", bufs=2, space="PSUM") as ps_dense,
            tc.tile_pool(name="ps_tr", bufs=2, space="PSUM") as ps_tr,
            tc.tile_pool(name="ps_pool", bufs=1, space="PSUM") as ps_pool,
            tc.tile_pool(name="ps_cnt", bufs=1, space="PSUM") as ps_cnt,
            tc.tile_pool(name="small", bufs=2) as small,
        ):
            # ---- constants & metadata
            idx_lo = meta.tile([P, max(nt_lo, 1) * 8], I16)
            idx_hi = meta.tile([P, max(nt_hi, 1) * 8], I16)
            dstoff = meta.tile([P, nt_lo + nt_hi], F32)
            gid_t = meta.tile([P, NB], F32)
            iota_t = meta.tile([P, P], F32)
            giota_t = meta.tile([P, NG], F32)
            ident_t = meta.tile([P, P], F32)
            ones_t = meta.tile([P, 1], F32)
            ones_bf = meta.tile([P, 1], BF16)
            xT_t = meta.tile([NODE_F, SLICE], F32)
            wemb_t = meta.tile([NODE_F, H], F32)
            bemb_t = meta.tile([P, 1], F32)
            wl_t = meta.tile([H, NL * H], F32)
            wr_t = meta.tile([H, NL * H], F32)
            bl_t = meta.tile([P, NL], F32)
            gs_t = meta.tile([P, NL], F32)
            bt_t = meta.tile([P, NL], F32)
            uT_t = meta.tile([16, NG], F32)
            wg_t = meta.tile([16, H], F32)
            bg_t = meta.tile([P, 1], F32)
            wf1a_t = meta.tile([H, H], F32)
            wf1b_t = meta.tile([H, H], F32)
            bf1_t = meta.tile([P, 1], F32)
            wf2_t = meta.tile([H, 2], F32)
            bf2_t = meta.tile([2, 1], F32)
            for t_, i_ in (
                (idx_lo, idx_lo_in), (idx_hi, idx_hi_in), (dstoff, dstoff_in),
                (gid_t, gid_in), (iota_t, iota_in), (giota_t, giota_in),
                (ident_t, ident_in), (ones_t, ones_in), (xT_t, xT_in),
                (wemb_t, wemb_in), (bemb_t, bemb_in), (wl_t, wl_in),
                (wr_t, wr_in), (bl_t, bl_in), (gs_t, gs_in), (bt_t, bt_in),
                (uT_t, uT_in), (wg_t, wg_in), (bg_t, bg_in),
                (wf1a_t, wf1_in[:H, :]), (wf1b_t, wf1_in[H:, :]),
                (bf1_t, bf1_in), (wf2_t, wf2_in), (bf2_t, bf2_in),
            ):
                nc.sync.dma_start(t_[:], i_[:])

            nc.vector.tensor_copy(ones_bf[:], ones_t[:])
            iota_bf = meta.tile([P, P], BF16)
            nc.vector.tensor_copy(iota_bf[:], iota_t[:])
            hT_a = hbuf.tile([P, SLICE], F32, name="hT_a")
            hT_b = hbuf.tile([P, SLICE], F32, name="hT_b")
            inv_bc = hbuf.tile([P, SLICE], F32, name="inv_bc")

            slices = [dram.tile([SLICE, H], BF16, name=f"slice{i}") for i in range(NL)]
            tables = [dram.tile([PADN, H], BF16, addr_space="Shared", name=f"table{i}")
                      for i in range(NL)]
            inv_row_d = dram.tile([NB, P], F32, name="inv_row_d")
            invg_row_d = dram.tile([1, NG], F32, name="invg_row_d")
            payload = dram.tile([P + 1, NG], F32, name="payload")
            payload_all = dram.tile([NCORES * (P + 1), NG], F32, addr_space="Shared",
                                    name="payload_all")

            def build_S(gidx):
                s = ohp.tile([P, P], BF16, tag="s")
                nc.vector.tensor_scalar(
                    out=s[:], in0=iota_bf[:],
                    scalar1=dstoff[:, gidx:gidx + 1], scalar2=None,
                    op0=AluOp.is_equal,
                )
                return s

            # ---- embed + degree counts ------------------------------------
            _sc_embed = nc.enter_named_scope("embed", False)
            for b in range(NB):
                hp = ps_dense.tile([P, P], F32, tag="d")
                nc.tensor.matmul(out=hp[:], lhsT=wemb_t[:],
                                 rhs=xT_t[:, b * P:(b + 1) * P],
                                 start=True, stop=True)
                nc.scalar.activation(hT_a[:, b * P:(b + 1) * P], hp[:],
                                     Act.Lrelu, bias=bemb_t[:], scale=1.0,
                                     alpha=NEG_SLOPE)
                cp = ps_cnt.tile([1, NG], F32, tag="c")
                refs = block_refs[b]
                for i_r, (st, ch, slot, g) in enumerate(refs):
                    s = build_S(g)
                    nc.tensor.matmul(out=cp[:1, :P], lhsT=ones_bf[:], rhs=s[:],
                                     start=(i_r == 0),
                                     stop=(i_r == len(refs) - 1))
                crow = small.tile([1, P], F32, tag="crow")
                nc.vector.tensor_scalar(out=crow[:], in0=cp[:1, :P],
                                        scalar1=1.0, scalar2=None,
                                        op0=AluOp.max)
                nc.vector.reciprocal(crow[:], crow[:])
                nc.sync.dma_start(inv_row_d[b:b + 1, :], crow[:])
                nc.sync.dma_start(
                    inv_bc[:, b * P:(b + 1) * P],
                    inv_row_d[b:b + 1, :].to_broadcast((P, P)))
                tp = ps_tr.tile([P, P], F32, tag="t")
                nc.tensor.transpose(out=tp[:], in_=hT_a[:, b * P:(b + 1) * P],
                                    identity=ident_t[:])
                stg = stgp.tile([P, P], BF16, tag="st")
                nc.vector.tensor_copy(stg[:], tp[:])
                nc.sync.dma_start(slices[0][b * P:(b + 1) * P, :], stg[:])

            nc.gpsimd.collective_compute(
                "AllGather", AluOp.bypass, replica_groups=RG,
                ins=[slices[0][:]], outs=[tables[0][:]],
            )
            nc.leave_named_scope("embed", _sc_embed[0], False)

            # ---- SAGE layers ---------------------------------------------
            hT_prev, hT_new = hT_a, hT_b
            pool_ps = ps_pool.tile([P, NG], F32, tag="pp")
            gcnt_ps = ps_cnt.tile([1, NG], F32, tag="c")

            for li in range(NL):
                _sc_l = nc.enter_named_scope(f"layer{li}", False)
                table_prev = tables[li]
                chunk_tiles = {0: [], 1: []}
                qrr = 0
                sched = []
                for h, idx_t_, pool_h in ((0, idx_lo, glo), (1, idx_hi, ghi)):
                    c0 = 0
                    for ntc in chunks[h]:
                        sched.append((h, idx_t_, pool_h, ntc, c0))
                        c0 += ntc
                # interleave lo/hi so each block's tiles arrive together
                lo_s = [e for e in sched if e[0] == 0]
                hi_s = [e for e in sched if e[0] == 1]
                inter = []
                i = j = 0
                while i < len(lo_s) or j < len(hi_s):
                    if i < len(lo_s):
                        inter.append(lo_s[i]); i += 1
                    if j < len(hi_s):
                        inter.append(hi_s[j]); j += 1
                for h, idx_t_, pool_h, ntc, c0 in inter:
                    g = pool_h.tile([P, ntc, P], BF16, tag=f"g{h}")
                    nidx = ntc * P
                    fs = nidx // 16
                    f0 = c0 * P // 16
                    nc.gpsimd.dma_gather(
                        out_ap=g[:],
                        in_ap=table_prev[h * HALF:(h + 1) * HALF, :],
                        idxs_ap=idx_t_[:, f0:f0 + fs],
                        num_idxs=nidx, num_idxs_reg=nidx, elem_size=H,
                        queue_num=qrr % 4, single_packet=False,
                    )
                    qrr += 1
                    chunk_tiles[h].append(g)

                for b in range(NB):
                    sp = ps_scat.tile([P, P], F32, tag="sc")
                    refs = block_refs[b]
                    for i_r, (st, ch, slot, g) in enumerate(refs):
                        s = build_S(g)
                        xs = chunk_tiles[st][ch][:, slot, :]
                        nc.tensor.matmul(out=sp[:], lhsT=xs, rhs=s[:],
                                         start=(i_r == 0),
                                         stop=(i_r == len(refs) - 1))
                    mt = meanp.tile([P, P], F32, tag="m")
                    nc.vector.tensor_tensor(
                        out=mt[:], in0=sp[:],
                        in1=inv_bc[:, b * P:(b + 1) * P], op=AluOp.mult)
                    hp = ps_dense.tile([P, P], F32, tag="d")
                    nc.tensor.matmul(out=hp[:],
                                     lhsT=wl_t[:, li * H:(li + 1) * H],
                                     rhs=mt[:], start=True, stop=False)
                    nc.tensor.matmul(out=hp[:],
                                     lhsT=wr_t[:, li * H:(li + 1) * H],
                                     rhs=hT_prev[:, b * P:(b + 1) * P],
                                     start=False, stop=True)
                    pre = prep_.tile([P, P], F32, tag="p")
                    nc.scalar.activation(pre[:], hp[:], Act.Lrelu,
                                         bias=bl_t[:, li:li + 1], scale=1.0,
                                         alpha=NEG_SLOPE)
                    nc.vector.tensor_scalar(
                        out=hT_new[:, b * P:(b + 1) * P], in0=pre[:],
                        scalar1=gs_t[:, li:li + 1], scalar2=bt_t[:, li:li + 1],
                        op0=AluOp.mult, op1=AluOp.add)
                    tp = ps_tr.tile([P, P], F32, tag="t")
                    nc.tensor.transpose(out=tp[:],
                                        in_=hT_new[:, b * P:(b + 1) * P],
                                        identity=ident_t[:])
                    stg = stgp.tile([P, P], BF16, tag="st")
                    nc.vector.tensor_copy(stg[:], tp[:])
                    if li < NL - 1:
                        nc.sync.dma_start(slices[li + 1][b * P:(b + 1) * P, :],
                                          stg[:])
                    else:
                        gb = ohp.tile([P, NG], BF16, tag="gb")
                        nc.vector.tensor_scalar(
                            out=gb[:], in0=giota_t[:],
                            scalar1=gid_t[:, b:b + 1], scalar2=None,
                            op0=AluOp.is_equal)
                        nc.tensor.matmul(out=pool_ps[:], lhsT=stg[:], rhs=gb[:],
                                         start=(b == 0), stop=(b == NB - 1))
                        nc.tensor.matmul(out=gcnt_ps[:], lhsT=ones_bf[:],
                                         rhs=gb[:],
                                         start=(b == 0), stop=(b == NB - 1))

                if li < NL - 1:
                    nc.gpsimd.collective_compute(
                        "AllGather", AluOp.bypass, replica_groups=RG,
                        ins=[slices[li + 1][:]], outs=[tables[li + 1][:]],
                    )
                hT_prev, hT_new = hT_new, hT_prev
                nc.leave_named_scope(f"layer{li}", _sc_l[0], False)

            # ---- pooling epilogue ----------------------------------------
            _sc_e = nc.enter_named_scope("epilogue", False)
            poolT = small.tile([P, NG], F32, tag="poolT")
            nc.vector.tensor_copy(poolT[:], pool_ps[:])
            gcrow = small.tile([1, NG], F32, tag="gcrow")
            nc.vector.tensor_copy(gcrow[:], gcnt_ps[:])
            nc.sync.dma_start(payload[:P, :], poolT[:])
            nc.sync.dma_start(payload[P:P + 1, :], gcrow[:])
            nc.gpsimd.collective_compute(
                "AllGather", AluOp.bypass, replica_groups=RG,
                ins=[payload[:]], outs=[payload_all[:]],
            )
            pool_acc = small.tile([P, NG], F32, tag="pacc")
            gc_acc = small.tile([1, NG], F32, tag="gacc")
            tmp = small.tile([P, NG], F32, tag="ptmp")
            tmpr = small.tile([1, NG], F32, tag="rtmp")
            for r in range(NCORES):
                base_r = r * (P + 1)
                if r == 0:
                    nc.sync.dma_start(pool_acc[:],
                                      payload_all[base_r:base_r + P, :])
                    nc.sync.dma_start(
                        gc_acc[:], payload_all[base_r + P:base_r + P + 1, :])
                else:
                    nc.sync.dma_start(tmp[:],
                                      payload_all[base_r:base_r + P, :])
                    nc.sync.dma_start(
                        tmpr[:], payload_all[base_r + P:base_r + P + 1, :])
                    nc.vector.tensor_add(pool_acc[:], pool_acc[:], tmp[:])
                    nc.vector.tensor_add(gc_acc[:], gc_acc[:], tmpr[:])
            nc.vector.tensor_scalar(out=gc_acc[:], in0=gc_acc[:], scalar1=1.0,
                                    scalar2=None, op0=AluOp.max)
            nc.vector.reciprocal(gc_acc[:], gc_acc[:])
            nc.sync.dma_start(invg_row_d[:], gc_acc[:])
            invg_bc = small.tile([P, NG], F32, tag="invgbc")
            nc.sync.dma_start(invg_bc[:],
                              invg_row_d[:1, :].to_broadcast((P, NG)))
            nc.vector.tensor_tensor(out=pool_acc[:], in0=pool_acc[:],
                                    in1=invg_bc[:], op=AluOp.mult)

            ug_ps = ps_dense.tile([P, NG], F32, tag="d")
            nc.tensor.matmul(out=ug_ps[:], lhsT=wg_t[:], rhs=uT_t[:],
                             start=True, stop=True)
            ugT = small.tile([P, NG], F32, tag="ugT")
            nc.scalar.activation(ugT[:], ug_ps[:], Act.Lrelu, bias=bg_t[:],
                                 scale=1.0, alpha=NEG_SLOPE)

            hid_ps = ps_dense.tile([P, NG], F32, tag="d")
            nc.tensor.matmul(out=hid_ps[:], lhsT=wf1a_t[:],
                             rhs=pool_acc[:], start=True, stop=False)
            nc.tensor.matmul(out=hid_ps[:], lhsT=wf1b_t[:], rhs=ugT[:],
                             start=False, stop=True)
            hidT = small.tile([P, NG], F32, tag="hidT")
            nc.scalar.activation(hidT[:], hid_ps[:], Act.Lrelu, bias=bf1_t[:],
                                 scale=1.0, alpha=NEG_SLOPE)

            y_ps = ps_tr.tile([2, NG], F32, tag="t")
            nc.tensor.matmul(out=y_ps[:], lhsT=wf2_t[:], rhs=hidT[:],
                             start=True, stop=True)
            yT = small.tile([2, NG], F32, tag="yT")
            nc.vector.tensor_scalar(out=yT[:], in0=y_ps[:], scalar1=bf2_t[:],
                                    scalar2=None, op0=AluOp.add)
            nc.sync.dma_start(y_out[:], yT[:])
            nc.leave_named_scope("epilogue", _sc_e[0], False)

    nc.finalize()
    _legalize_sync_waits(nc)

    common = dict(
        iota=iota_np, giota=giota_np, ident=ident_np, ones=ones_np,
        wemb=np.asarray(W_emb, np.float32),
        bemb=np.asarray(b_emb, np.float32).reshape(P, 1),
        wl=np.asarray(W_l, np.float32).transpose(1, 0, 2).reshape(H, NL * H).copy(),
        wr=np.asarray(W_r, np.float32).transpose(1, 0, 2).reshape(H, NL * H).copy(),
        bl=np.asarray(b_l, np.float32).T.copy(),
        gs=gscale.T.copy(), bt=beta_np.T.copy(),
        uT=u.T.copy(),
        wg=np.asarray(W_g, np.float32),
        bg=np.asarray(b_g, np.float32).reshape(P, 1),
        wf1=np.asarray(W_f1, np.float32),
        bf1=np.asarray(b_f1, np.float32).reshape(P, 1),
        wf2=np.asarray(W_f2, np.float32),
        bf2=np.asarray(b_f2, np.float32).reshape(2, 1),
    )
    in_maps = []
    for c in range(NCORES):
        cd = prep["core_data"][c]
        in_maps.append(dict(
            common,
            xT=xT[:, c * SLICE:(c + 1) * SLICE].copy(),
            idx_lo=cd["idx_lo"], idx_hi=cd["idx_hi"],
            dstoff=cd["dstoff"], gid=prep["gids"][c],
        ))

    res = run_bass_kernel_spmd(nc, in_maps, core_ids=list(range(NCORES)),
                               trace=TRACE)
    global LAST_RESULT
    LAST_RESULT = res
    return np.asarray(res.results[0]["y"]).T.astype(np.float32).copy()


TRACE = False
LAST_RESULT = None



# revision 7
# speedup vs baseline: 1.3324x; 1.3324x over previous
"""Trainium2 Bass kernel for MaterialsGraphSAGE (4-layer GraphSAGE + pooling).

Strategy (8 NeuronCores, one chip):
- Node space padded to 50176 = 8 x 6272; core c owns nodes [6272c, 6272(c+1)).
- Edges are owned by their dst core, grouped per 128-node dst block, split by
  src half (dma_gather idx is int16, so the h table is addressed as two 25088
  row halves), padded to 128-edge tiles. Tile counts per (block, half) are
  normalized to the max across cores so the SPMD program structure is
  core-independent; only the idx / S data differs per core.
- The scatter-mean matrices S[e, dst] (one-hot scaled by 1/deg[dst]) are
  precomputed on the host from edge_index and streamed from DRAM per layer,
  so no per-layer one-hot construction happens on device.
- Per layer: dma_gather preps (prepare_only + trigger_dma) fetch h[src] rows;
  the scatter-mean is a matmul against the streamed S accumulated in PSUM
  (transposed: meanT[f, n]); the dense SAGE update + BN runs in transposed
  layout so per-channel affine ops are per-partition. Each core's new h slice
  is written to DRAM and AllGathered into a pair-shared full table.
- Final layer accumulates graph pooling (one-hot over graph ids) + counts;
  contributions ride a small AllReduce; every core computes the tiny final
  MLP; core 0's output is returned.
"""

import sys

for _p in ("/opt/trn_rl_repo",):
    if _p not in sys.path:
        sys.path.insert(0, _p)

import ml_dtypes
import numpy as np

import concourse.bacc as bacc
import concourse.mybir as mybir
import concourse.tile as tile
from concourse.bass_utils import run_bass_kernel_spmd
from concourse.vector_clock import ScopedClock

F32 = mybir.dt.float32
BF16 = mybir.dt.bfloat16
I16 = mybir.dt.int16

P = 128
NCORES = 8
NN = 50000
NG = 256
SLICE = 6272
PADN = SLICE * NCORES      # 50176
HALF = PADN // 2           # 25088
NB = SLICE // P            # 49 blocks per core
NL = 4                     # SAGE layers
H = 128
NODE_F = 64
CH_TILES = 16              # gather tiles per dma_gather call
NEG_SLOPE = 0.01
BN_EPS = 1e-5


# ---------------------------------------------------------------------------
# walrus in this container rejects >1 sync wait per instruction; split them.
def _patch_tile_drain():
    def _drain_and_barrier(self, tick_clock, wait_clock):
        drain_inst = self.nc.sync.drain()
        wait_clock.add_sem_waits(
            drain_inst.ins, ScopedClock({None: tick_clock.global_clock})
        )
        si = drain_inst.ins.sync_info
        waits = list(si.on_wait) if si is not None else []
        if len(waits) > 1:
            drain_inst.ins.sync_info = mybir.SyncInfo(
                on_wait=[waits[0]], on_update=list(si.on_update)
            )
            for w in waits[1:]:
                extra = self.nc.sync.drain()
                extra.ins.sync_info = mybir.SyncInfo(on_wait=[w], on_update=[])
        self.nc.all_engine_barrier()
        assert self.sems is not None
        popped = self.nc._tile_sem_poison_stack.pop()
        assert popped is self._sem_poison
        self.nc.clear_and_free_semaphores(list(self.sems.allocated().values()))
        self.nc.all_engine_barrier()

    tile.TileContext._drain_and_barrier = _drain_and_barrier


_patch_tile_drain()


def _legalize_sync_waits(nc, max_waits=1):
    for fn in nc.m.functions:
        for bb in fn.blocks:
            out = []
            changed = False
            for ins in bb.instructions:
                si = ins.sync_info
                if si is not None and len(si.on_wait) > max_waits:
                    waits = list(si.on_wait)
                    for w in waits[:-max_waits]:
                        nop = mybir.InstNoOp(
                            name=f"WSPLIT-{nc.next_id()}", ins=[], outs=[]
                        )
                        nop.engine = ins.engine
                        nop.sync_info = mybir.SyncInfo(on_wait=[w], on_update=[])
                        out.append(nop)
                    ins.sync_info = mybir.SyncInfo(
                        on_wait=waits[-max_waits:], on_update=list(si.on_update)
                    )
                    changed = True
                out.append(ins)
            if changed:
                bb.instructions = out


# ---------------------------------------------------------------------------
def _wrap_idx(flat):
    """int16 row indices -> dma_gather idx buffer [128, n/16] (wrapped in 16
    partitions, replicated across the 8 Q7 core groups)."""
    n = flat.shape[0]
    assert n % 16 == 0
    buf = np.zeros((P, n // 16), np.int16)
    j = np.arange(n)
    for k in range(8):
        buf[16 * k + (j % 16), j // 16] = flat
    return buf


def _prepare(src, dst, batch_gid):
    """Group edges per core / dst block / src half; normalize tile counts
    across cores so all cores share one program structure. Precompute the
    scatter matrices S[e, dst] = 1/deg[dst] (one tile per 128-edge group)."""
    deg = np.bincount(dst, minlength=NN).astype(np.float32)
    inv_deg = (1.0 / np.maximum(deg, 1.0)).astype(np.float32)

    per_core = []
    for c in range(NCORES):
        base = c * SLICE
        m = (dst >= base) & (dst < base + SLICE)
        s = src[m]
        d = dst[m]
        blk = (d - base) >> 7
        half = (s >= HALF).astype(np.int64)
        key = blk * 2 + half
        order = np.argsort(key, kind="stable")
        s, d, key = s[order], d[order], key[order]
        bounds = np.searchsorted(key, np.arange(2 * NB + 1))
        cells = {}
        for b in range(NB):
            for h in (0, 1):
                lo, hi = bounds[2 * b + h], bounds[2 * b + h + 1]
                if hi > lo:
                    sl = (s[lo:hi] - (HALF if h else 0)).astype(np.int16)
                    doff = (d[lo:hi] - base - b * P).astype(np.int64)
                    vals = inv_deg[d[lo:hi]]
                    cells[(b, h)] = (sl, doff, vals)
        per_core.append(cells)

    # normalized tile counts
    NT = np.zeros((NB, 2), np.int64)
    for b in range(NB):
        for h in (0, 1):
            n = max((len(per_core[c].get((b, h), ((), (), ()))[0])
                     for c in range(NCORES)), default=0)
            NT[b, h] = -(-n // P)
        if NT[b].sum() == 0:
            NT[b, 0] = 1

    nt_lo = int(NT[:, 0].sum())
    nt_hi = int(NT[:, 1].sum())
    nt_tot = nt_lo + nt_hi

    # shared structure: stream positions and block refs
    pos = {0: 0, 1: 0}
    block_refs = [[] for _ in range(NB)]
    tile_pos = {}              # (b,h,t) -> (stream, stream_pos, gidx)
    for b in range(NB):
        for h in (0, 1):
            for t in range(int(NT[b, h])):
                p_ = pos[h]
                g = p_ if h == 0 else nt_lo + p_
                block_refs[b].append((h, p_ // CH_TILES, p_ % CH_TILES, g))
                tile_pos[(b, h, t)] = (h, p_, g)
                pos[h] += 1

    # chunk sizes per stream (last may be partial)
    chunks = {}
    for h, nt in ((0, nt_lo), (1, nt_hi)):
        chunks[h] = [min(CH_TILES, nt - c0) for c0 in range(0, nt, CH_TILES)]

    # per-core data buffers
    core_data = []
    rowsel = np.arange(P)
    for c in range(NCORES):
        flat = {0: np.zeros(nt_lo * P, np.int16),
                1: np.zeros(nt_hi * P, np.int16)}
        S = np.zeros((P, nt_tot, P), np.float32)
        for b in range(NB):
            for h in (0, 1):
                sl, doff, vals = per_core[c].get(
                    (b, h), (np.zeros(0, np.int16), np.zeros(0, np.int64),
                             np.zeros(0, np.float32)))
                n = len(sl)
                for t in range(int(NT[b, h])):
                    _, p_, g = tile_pos[(b, h, t)]
                    seg_s = sl[t * P:(t + 1) * P]
                    seg_d = doff[t * P:(t + 1) * P]
                    seg_v = vals[t * P:(t + 1) * P]
                    flat[h][p_ * P:p_ * P + len(seg_s)] = seg_s
                    if len(seg_d):
                        S[rowsel[:len(seg_d)], g, seg_d] = seg_v
        core_data.append(dict(
            idx_lo=_wrap_idx(flat[0]) if nt_lo else np.zeros((P, 8), np.int16),
            idx_hi=_wrap_idx(flat[1]) if nt_hi else np.zeros((P, 8), np.int16),
            stab=S.reshape(P, nt_tot * P).astype(ml_dtypes.bfloat16),
        ))

    # graph id per node column layout [128, NB] (pad nodes -> -1)
    gids = []
    for c in range(NCORES):
        base = c * SLICE
        col = np.full(SLICE, -1.0, np.float32)
        npad = min(max(NN - base, 0), SLICE)
        if npad > 0:
            col[:npad] = batch_gid[base:base + npad]
        gids.append(col.reshape(NB, P).T.copy())

    return dict(nt_lo=nt_lo, nt_hi=nt_hi, chunks=chunks,
                block_refs=block_refs, core_data=core_data, gids=gids)


def kernel(x, edge_index, u, batch, W_emb, b_emb, W_l, b_l, W_r, gamma, beta,
           W_g, b_g, W_f1, b_f1, W_f2, b_f2):
    x = np.asarray(x, np.float32)
    edge_index = np.asarray(edge_index)
    u = np.asarray(u, np.float32)
    batch = np.asarray(batch)

    src = edge_index[0].astype(np.int64)
    dst = edge_index[1].astype(np.int64)
    prep = _prepare(src, dst, batch.astype(np.float32))

    nt_lo, nt_hi = prep["nt_lo"], prep["nt_hi"]
    nt_tot = nt_lo + nt_hi
    chunks = prep["chunks"]
    block_refs = prep["block_refs"]

    xT = np.zeros((NODE_F, PADN), np.float32)
    xT[:, :NN] = x.T

    inv_std = np.float32(1.0 / np.sqrt(1.0 + BN_EPS))
    gscale = np.asarray(gamma, np.float32) * inv_std  # [L, H]
    beta_np = np.asarray(beta, np.float32)

    giota_np = np.broadcast_to(np.arange(NG, dtype=np.float32), (P, NG)).copy()
    ident_np = np.eye(P, dtype=np.float32)
    ones_np = np.ones((P, 1), np.float32)

    # ------------------------------------------------------------------
    nc = bacc.Bacc(None, num_swdge_queues=4)

    def din(name, shape, dtype=F32):
        return nc.dram_tensor(name, shape, dtype, kind="ExternalInput")

    xT_in = din("xT", [NODE_F, SLICE])
    idx_lo_in = din("idx_lo", [P, max(nt_lo, 1) * 8], I16)
    idx_hi_in = din("idx_hi", [P, max(nt_hi, 1) * 8], I16)
    stab_in = din("stab", [P, nt_tot * P], BF16)
    gid_in = din("gid", [P, NB])
    giota_in = din("giota", [P, NG])
    ident_in = din("ident", [P, P])
    ones_in = din("ones", [P, 1])
    wemb_in = din("wemb", [NODE_F, H])
    bemb_in = din("bemb", [P, 1])
    wl_in = din("wl", [H, NL * H])
    wr_in = din("wr", [H, NL * H])
    bl_in = din("bl", [P, NL])
    gs_in = din("gs", [P, NL])
    bt_in = din("bt", [P, NL])
    uT_in = din("uT", [16, NG])
    wg_in = din("wg", [16, H])
    bg_in = din("bg", [P, 1])
    wf1_in = din("wf1", [2 * H, H])
    bf1_in = din("bf1", [P, 1])
    wf2_in = din("wf2", [H, 2])
    bf2_in = din("bf2", [2, 1])
    y_out = nc.dram_tensor("y", [2, NG], F32, kind="ExternalOutput")

    RG = [list(range(NCORES))]
    AluOp = mybir.AluOpType
    Act = mybir.ActivationFunctionType

    n_sch = -(-nt_tot // CH_TILES)   # S stream chunks
    sch_sizes = [min(CH_TILES, nt_tot - c0)
                 for c0 in range(0, nt_tot, CH_TILES)]

    with tile.TileContext(nc) as tc:
        with (
            tc.tile_pool(name="dram", bufs=1, space="DRAM") as dram,
            tc.tile_pool(name="meta", bufs=1) as meta,
            tc.tile_pool(name="hbuf", bufs=1) as hbuf,
            tc.tile_pool(name="glo", bufs=3) as glo,
            tc.tile_pool(name="ghi", bufs=3) as ghi,
            tc.tile_pool(name="spool", bufs=3) as spool,
            tc.tile_pool(name="oh", bufs=4) as ohp,
            tc.tile_pool(name="mean", bufs=3) as meanp,
            tc.tile_pool(name="pre", bufs=3) as prep_,
            tc.tile_pool(name="stg", bufs=3) as stgp,
            tc.tile_pool(name="ps_scat", bufs=2, space="PSUM") as ps_scat,
            tc.tile_pool(name="ps_dense", bufs=2, space="PSUM") as ps_dense,
            tc.tile_pool(name="ps_tr", bufs=2, space="PSUM") as ps_tr,
            tc.tile_pool(name="ps_pool", bufs=1, space="PSUM") as ps_pool,
            tc.tile_pool(name="ps_cnt", bufs=1, space="PSUM") as ps_cnt,
            tc.tile_pool(name="small", bufs=2) as small,
        ):
            # ---- constants & metadata
            idx_lo = meta.tile([P, max(nt_lo, 1) * 8], I16)
            idx_hi = meta.tile([P, max(nt_hi, 1) * 8], I16)
            gid_t = meta.tile([P, NB], F32)
            giota_t = meta.tile([P, NG], F32)
            ident_t = meta.tile([P, P], F32)
            ones_t = meta.tile([P, 1], F32)
            ones_bf = meta.tile([P, 1], BF16)
            xT_t = meta.tile([NODE_F, SLICE], F32)
            wemb_t = meta.tile([NODE_F, H], F32)
            bemb_t = meta.tile([P, 1], F32)
            wl_t = meta.tile([H, NL * H], F32)
            wr_t = meta.tile([H, NL * H], F32)
            bl_t = meta.tile([P, NL], F32)
            gs_t = meta.tile([P, NL], F32)
            bt_t = meta.tile([P, NL], F32)
            uT_t = meta.tile([16, NG], F32)
            wg_t = meta.tile([16, H], F32)
            bg_t = meta.tile([P, 1], F32)
            wf1a_t = meta.tile([H, H], F32)
            wf1b_t = meta.tile([H, H], F32)
            bf1_t = meta.tile([P, 1], F32)
            wf2_t = meta.tile([H, 2], F32)
            bf2_t = meta.tile([2, 1], F32)
            for t_, i_ in (
                (idx_lo, idx_lo_in), (idx_hi, idx_hi_in),
                (gid_t, gid_in), (giota_t, giota_in),
                (ident_t, ident_in), (ones_t, ones_in), (xT_t, xT_in),
                (wemb_t, wemb_in), (bemb_t, bemb_in), (wl_t, wl_in),
                (wr_t, wr_in), (bl_t, bl_in), (gs_t, gs_in), (bt_t, bt_in),
                (uT_t, uT_in), (wg_t, wg_in), (bg_t, bg_in),
                (wf1a_t, wf1_in[:H, :]), (wf1b_t, wf1_in[H:, :]),
                (bf1_t, bf1_in), (wf2_t, wf2_in), (bf2_t, bf2_in),
            ):
                nc.sync.dma_start(t_[:], i_[:])

            nc.vector.tensor_copy(ones_bf[:], ones_t[:])
            hT_a = hbuf.tile([P, SLICE], F32, name="hT_a")
            hT_b = hbuf.tile([P, SLICE], F32, name="hT_b")

            slices = [dram.tile([SLICE, H], BF16, name=f"slice{i}") for i in range(NL)]
            tables = [dram.tile([PADN, H], BF16, addr_space="Shared", name=f"table{i}")
                      for i in range(NL)]
            payload = dram.tile([P + 1, NG], F32, name="payload")

            # ---- embed ----------------------------------------------------
            _sc_embed = nc.enter_named_scope("embed", False)
            for b in range(NB):
                hp = ps_dense.tile([P, P], F32, tag="d")
                nc.tensor.matmul(out=hp[:], lhsT=wemb_t[:],
                                 rhs=xT_t[:, b * P:(b + 1) * P],
                                 start=True, stop=True)
                nc.scalar.activation(hT_a[:, b * P:(b + 1) * P], hp[:],
                                     Act.Lrelu, bias=bemb_t[:], scale=1.0,
                                     alpha=NEG_SLOPE)
                tp = ps_tr.tile([P, P], F32, tag="t")
                nc.tensor.transpose(out=tp[:], in_=hT_a[:, b * P:(b + 1) * P],
                                    identity=ident_t[:])
                stg = stgp.tile([P, P], BF16, tag="st")
                nc.vector.tensor_copy(stg[:], tp[:])
                nc.sync.dma_start(slices[0][b * P:(b + 1) * P, :], stg[:])

            nc.gpsimd.collective_compute(
                "AllGather", AluOp.bypass, replica_groups=RG,
                ins=[slices[0][:]], outs=[tables[0][:]],
            )
            nc.leave_named_scope("embed", _sc_embed[0], False)

            # ---- SAGE layers ---------------------------------------------
            hT_prev, hT_new = hT_a, hT_b
            pool_ps = ps_pool.tile([P, NG], F32, tag="pp")
            gcnt_ps = ps_cnt.tile([1, NG], F32, tag="c")

            for li in range(NL):
                _sc_l = nc.enter_named_scope(f"layer{li}", False)
                table_prev = tables[li]

                # gather preps + triggers (interleave lo/hi)
                sched = []
                for h, idx_t_, pool_h in ((0, idx_lo, glo), (1, idx_hi, ghi)):
                    c0 = 0
                    for ntc in chunks[h]:
                        sched.append((h, idx_t_, pool_h, ntc, c0))
                        c0 += ntc
                lo_s = [e for e in sched if e[0] == 0]
                hi_s = [e for e in sched if e[0] == 1]
                inter = []
                i = j = 0
                while i < len(lo_s) or j < len(hi_s):
                    if i < len(lo_s):
                        inter.append(lo_s[i]); i += 1
                    if j < len(hi_s):
                        inter.append(hi_s[j]); j += 1
                chunk_tiles = {0: [], 1: []}
                qrr = 0
                for h, idx_t_, pool_h, ntc, c0 in inter:
                    g = pool_h.tile([P, ntc, P], BF16, tag=f"g{h}")
                    nidx = ntc * P
                    fs = nidx // 16
                    f0 = c0 * P // 16
                    nc.gpsimd.dma_gather(
                        out_ap=g[:],
                        in_ap=table_prev[h * HALF:(h + 1) * HALF, :],
                        idxs_ap=idx_t_[:, f0:f0 + fs],
                        num_idxs=nidx, num_idxs_reg=nidx, elem_size=H,
                        queue_num=qrr % 4, single_packet=False,
                    )
                    qrr += 1
                    chunk_tiles[h].append(g)

                # S stream for this layer (same lo/hi interleave as gathers
                # so buffer recycling follows block consumption order)
                s_tiles = {0: [], 1: []}
                for h, idx_t_, pool_h, ntc, c0 in inter:
                    g0 = c0 if h == 0 else nt_lo + c0
                    st_ = spool.tile([P, ntc * P], BF16, tag=f"s{h}")
                    nc.sync.dma_start(
                        st_[:], stab_in[:, g0 * P:(g0 + ntc) * P])
                    s_tiles[h].append(st_)

                for b in range(NB):
                    sp = ps_scat.tile([P, P], F32, tag="sc")
                    refs = block_refs[b]
                    for i_r, (st, ch, slot, g) in enumerate(refs):
                        xs = chunk_tiles[st][ch][:, slot, :]
                        sref = s_tiles[st][ch][:, slot * P:(slot + 1) * P]
                        nc.tensor.matmul(out=sp[:], lhsT=xs, rhs=sref,
                                         start=(i_r == 0),
                                         stop=(i_r == len(refs) - 1))
                    mt = meanp.tile([P, P], F32, tag="m")
                    nc.vector.tensor_copy(mt[:], sp[:])
                    hp = ps_dense.tile([P, P], F32, tag="d")
                    nc.tensor.matmul(out=hp[:],
                                     lhsT=wl_t[:, li * H:(li + 1) * H],
                                     rhs=mt[:], start=True, stop=False)
                    nc.tensor.matmul(out=hp[:],
                                     lhsT=wr_t[:, li * H:(li + 1) * H],
                                     rhs=hT_prev[:, b * P:(b + 1) * P],
                                     start=False, stop=True)
                    pre = prep_.tile([P, P], F32, tag="p")
                    nc.scalar.activation(pre[:], hp[:], Act.Lrelu,
                                         bias=bl_t[:, li:li + 1], scale=1.0,
                                         alpha=NEG_SLOPE)
                    nc.vector.tensor_scalar(
                        out=hT_new[:, b * P:(b + 1) * P], in0=pre[:],
                        scalar1=gs_t[:, li:li + 1], scalar2=bt_t[:, li:li + 1],
                        op0=AluOp.mult, op1=AluOp.add)
                    tp = ps_tr.tile([P, P], F32, tag="t")
                    nc.tensor.transpose(out=tp[:],
                                        in_=hT_new[:, b * P:(b + 1) * P],
                                        identity=ident_t[:])
                    stg = stgp.tile([P, P], BF16, tag="st")
                    nc.vector.tensor_copy(stg[:], tp[:])
                    if li < NL - 1:
                        nc.sync.dma_start(slices[li + 1][b * P:(b + 1) * P, :],
                                          stg[:])
                    else:
                        gb = ohp.tile([P, NG], BF16, tag="gb")
                        nc.vector.tensor_scalar(
                            out=gb[:], in0=giota_t[:],
                            scalar1=gid_t[:, b:b + 1], scalar2=None,
                            op0=AluOp.is_equal)
                        nc.tensor.matmul(out=pool_ps[:], lhsT=stg[:], rhs=gb[:],
                                         start=(b == 0), stop=(b == NB - 1))
                        nc.tensor.matmul(out=gcnt_ps[:], lhsT=ones_bf[:],
                                         rhs=gb[:],
                                         start=(b == 0), stop=(b == NB - 1))

                if li < NL - 1:
                    nc.gpsimd.collective_compute(
                        "AllGather", AluOp.bypass, replica_groups=RG,
                        ins=[slices[li + 1][:]], outs=[tables[li + 1][:]],
                    )
                hT_prev, hT_new = hT_new, hT_prev
                nc.leave_named_scope(f"layer{li}", _sc_l[0], False)

            # ---- pooling epilogue ----------------------------------------
            _sc_e = nc.enter_named_scope("epilogue", False)
            poolT = small.tile([P, NG], F32, tag="poolT")
            nc.vector.tensor_copy(poolT[:], pool_ps[:])
            gcrow = small.tile([1, NG], F32, tag="gcrow")
            nc.vector.tensor_copy(gcrow[:], gcnt_ps[:])
            nc.sync.dma_start(payload[:P, :], poolT[:])
            nc.sync.dma_start(payload[P:P + 1, :], gcrow[:])
            nc.gpsimd.collective_compute(
                "AllReduce", AluOp.add, replica_groups=RG,
                ins=[payload[:]], outs=[payload[:]],
            )
            pool_acc = small.tile([P, NG], F32, tag="pacc")
            gc_acc = small.tile([1, NG], F32, tag="gacc")
            nc.sync.dma_start(pool_acc[:], payload[:P, :])
            nc.sync.dma_start(gc_acc[:], payload[P:P + 1, :])
            nc.vector.tensor_scalar(out=gc_acc[:], in0=gc_acc[:], scalar1=1.0,
                                    scalar2=None, op0=AluOp.max)
            nc.vector.reciprocal(gc_acc[:], gc_acc[:])
            invg_row_d = dram.tile([1, NG], F32, name="invg_row_d")
            nc.sync.dma_start(invg_row_d[:], gc_acc[:])
            invg_bc = small.tile([P, NG], F32, tag="invgbc")
            nc.sync.dma_start(invg_bc[:],
                              invg_row_d[:1, :].to_broadcast((P, NG)))
            nc.vector.tensor_tensor(out=pool_acc[:], in0=pool_acc[:],
                                    in1=invg_bc[:], op=AluOp.mult)

            ug_ps = ps_dense.tile([P, NG], F32, tag="d")
            nc.tensor.matmul(out=ug_ps[:], lhsT=wg_t[:], rhs=uT_t[:],
                             start=True, stop=True)
            ugT = small.tile([P, NG], F32, tag="ugT")
            nc.scalar.activation(ugT[:], ug_ps[:], Act.Lrelu, bias=bg_t[:],
                                 scale=1.0, alpha=NEG_SLOPE)

            hid_ps = ps_dense.tile([P, NG], F32, tag="d")
            nc.tensor.matmul(out=hid_ps[:], lhsT=wf1a_t[:],
                             rhs=pool_acc[:], start=True, stop=False)
            nc.tensor.matmul(out=hid_ps[:], lhsT=wf1b_t[:], rhs=ugT[:],
                             start=False, stop=True)
            hidT = small.tile([P, NG], F32, tag="hidT")
            nc.scalar.activation(hidT[:], hid_ps[:], Act.Lrelu, bias=bf1_t[:],
                                 scale=1.0, alpha=NEG_SLOPE)

            y_ps = ps_tr.tile([2, NG], F32, tag="t")
            nc.tensor.matmul(out=y_ps[:], lhsT=wf2_t[:], rhs=hidT[:],
                             start=True, stop=True)
            yT = small.tile([2, NG], F32, tag="yT")
            nc.vector.tensor_scalar(out=yT[:], in0=y_ps[:], scalar1=bf2_t[:],
                                    scalar2=None, op0=AluOp.add)
            nc.sync.dma_start(y_out[:], yT[:])
            nc.leave_named_scope("epilogue", _sc_e[0], False)

    nc.finalize()
    _legalize_sync_waits(nc)

    common = dict(
        giota=giota_np, ident=ident_np, ones=ones_np,
        wemb=np.asarray(W_emb, np.float32),
        bemb=np.asarray(b_emb, np.float32).reshape(P, 1),
        wl=np.asarray(W_l, np.float32).transpose(1, 0, 2).reshape(H, NL * H).copy(),
        wr=np.asarray(W_r, np.float32).transpose(1, 0, 2).reshape(H, NL * H).copy(),
        bl=np.asarray(b_l, np.float32).T.copy(),
        gs=gscale.T.copy(), bt=beta_np.T.copy(),
        uT=u.T.copy(),
        wg=np.asarray(W_g, np.float32),
        bg=np.asarray(b_g, np.float32).reshape(P, 1),
        wf1=np.asarray(W_f1, np.float32),
        bf1=np.asarray(b_f1, np.float32).reshape(P, 1),
        wf2=np.asarray(W_f2, np.float32),
        bf2=np.asarray(b_f2, np.float32).reshape(2, 1),
    )
    in_maps = []
    for c in range(NCORES):
        cd = prep["core_data"][c]
        in_maps.append(dict(
            common,
            xT=xT[:, c * SLICE:(c + 1) * SLICE].copy(),
            idx_lo=cd["idx_lo"], idx_hi=cd["idx_hi"],
            stab=cd["stab"], gid=prep["gids"][c],
        ))

    res = run_bass_kernel_spmd(nc, in_maps, core_ids=list(range(NCORES)),
                               trace=TRACE)
    global LAST_RESULT
    LAST_RESULT = res
    return np.asarray(res.results[0]["y"]).T.astype(np.float32).copy()


TRACE = False
LAST_RESULT = None
